# revision 9
# baseline (speedup 1.0000x reference)
"""MoE-routed conditional conv kernel for Trainium2 (8 NeuronCores).

Problem: x:[64,256,32,32], 4 conv branches (k=1,3,5,7) with per-sample
branch selection (sample_arc) and a per-sample class-embedding bias
(e_b[y]).  The reference computes all 4 branches for every sample and
masks; we route: each sample's conv is computed only for its selected
branch.

Distribution: SPMD over 8 cores, one identical program, per-core DATA
chosen by host-side routing.  Work unit = "half-slot" = (sample, half of
the 32 output rows).  Each core runs, for branch b with per-core slot
count n_b = ceil(2*count_b/8), n_b half-slots of that branch.  A
half-slot's conv is a sum over k*k taps x 2 cin-chunks of 128x128x512
matmuls accumulated in PSUM (2 cout chunks => 2 PSUM tiles), then a
per-partition bias add (embedding row) on Scalar/Vector engine, then DMA
out.  Dummy half-slots (zero input, output dropped) pad each branch's
count to a multiple of the core count.
"""

import math
import sys
import types

import numpy as np

try:
    import concourse.bass as bass  # noqa: F401
except Exception:  # pragma: no cover - fallback when env lacks preloaded paths
    for p in ("/opt/trn_rl_repo", "/root/.axon_site/_ro/trn_rl_repo"):
        if p not in sys.path:
            sys.path.insert(0, p)
    import concourse.bass as bass  # noqa: F401

import ml_dtypes
import concourse.tile as tile
from concourse import bacc, mybir
from concourse import bass_utils

N_CORES = 8
NUM_BRANCH = 4
KERNEL_SIZES = (1, 3, 5, 7)
IN_C = 256
OUT_C = 256
H = W = 32
HALF = 16  # output rows per half-slot

# compute dtype for matmul operands: "bf16" | "f32" | "f32r"
COMPUTE_DT = "bf16"
TAP_BLOCK = 8          # taps per streamed weight block
GROUP = 4              # half-slots per psum group (x2 oc = 8 psum banks)

_DT_MAP = {
    "bf16": (mybir.dt.bfloat16, ml_dtypes.bfloat16),
    "f32": (mybir.dt.float32, np.float32),
    "f32r": (mybir.dt.float32r, np.float32),
}

_PROGRAM_CACHE = {}


def _install_profile_hook():
    """Register the axon NTFF profile hook if the image's antenv lacks it."""
    name = "antenv.axon_hooks"
    if name in sys.modules:
        return
    try:
        import antenv.axon_hooks  # noqa: F401
        return
    except ImportError:
        pass
    m = types.ModuleType(name)
    holder = [None]
    m.set_axon_ntff_profile_hook = lambda h: holder.__setitem__(0, h)
    m.get_axon_ntff_profile_hook = lambda: holder[0]
    sys.modules[name] = m
    try:
        import antenv
        antenv.axon_hooks = m
        from trn_agent_boot.trn_boot import _ntff_profile_via_ctypes
        m.set_axon_ntff_profile_hook(
            _ntff_profile_via_ctypes("/opt/axon/libaxon_pjrt.so")
        )
    except Exception:
        pass


def _build_program(cfg, dt_key):
    """Build the SPMD Bass program for a slot config.

    cfg: tuple of (k, n_slots) per branch, n_slots = per-core half-slots.
    """
    key = (cfg, dt_key)
    if key in _PROGRAM_CACHE:
        return _PROGRAM_CACHE[key]

    mdt, _ = _DT_MAP[dt_key]
    nc = bacc.Bacc("TRN2", target_bir_lowering=False, debug=False,
                   num_devices=N_CORES)

    n_total = sum(n for _, n in cfg)

    x_d = {}
    w_d = {}
    for b, (k, n) in enumerate(cfg):
        if n == 0:
            continue
        c = k // 2
        rows, wp = HALF + 2 * c, W + 2 * c
        x_d[b] = nc.dram_tensor(f"x{b}", [128, n, 2, rows, wp], mdt,
                                kind="ExternalInput").ap()
        w_d[b] = nc.dram_tensor(f"w{b}", [128, k * k * 4 * 128], mdt,
                                kind="ExternalInput").ap()
    emb_d = nc.dram_tensor("emb", [128, n_total * 2], mybir.dt.float32,
                           kind="ExternalInput").ap()
    out_d = nc.dram_tensor("out", [n_total, 128, 1024], mybir.dt.float32,
                           kind="ExternalOutput").ap()

    from contextlib import ExitStack
    with tile.TileContext(nc) as tc:
        with ExitStack() as ctx:
            xpool = ctx.enter_context(tc.tile_pool(name="xpool", bufs=1))
            wpool = ctx.enter_context(tc.tile_pool(name="wpool", bufs=3))
            epool = ctx.enter_context(tc.tile_pool(name="epool", bufs=1))
            opool = ctx.enter_context(tc.tile_pool(name="opool", bufs=6))
            ppool = ctx.enter_context(
                tc.tile_pool(name="ppool", bufs=8, space="PSUM"))

            emb_t = epool.tile([128, n_total * 2], mybir.dt.float32, tag="emb")
            nc.scalar.dma_start(emb_t[:], emb_d[:])

            out_i = 0
            slot_base = 0
            for b, (k, n) in enumerate(cfg):
                if n == 0:
                    continue
                c = k // 2
                rows, wp = HALF + 2 * c, W + 2 * c
                k2 = k * k

                for g0 in range(0, n, GROUP):
                    gsl = list(range(g0, min(g0 + GROUP, n)))
                    ng = len(gsl)
                    xt = xpool.tile([128, ng, 2, rows, wp], mdt,
                                    tag=f"x{b}_{g0}")
                    nc.sync.dma_start(xt[:], x_d[b][:, g0:g0 + ng])
                    ps = {}
                    for i in gsl:
                        for oc in range(2):
                            ps[(i, oc)] = ppool.tile(
                                [128, 512], mybir.dt.float32,
                                tag="acc", name=f"acc_{b}_{g0}_{i}_{oc}")
                    for t0 in range(0, k2, TAP_BLOCK):
                        nt = min(TAP_BLOCK, k2 - t0)
                        wt = wpool.tile([128, nt * 4 * 128], mdt, tag="wblk")
                        nc.gpsimd.dma_start(
                            wt[:], w_d[b][:, t0 * 512:(t0 + nt) * 512])
                        for tt in range(nt):
                            t = t0 + tt
                            dy, dx = divmod(t, k)
                            for ic in range(2):
                                for oc in range(2):
                                    lhs = wt[:, ((tt * 2 + ic) * 2 + oc) * 128:
                                             ((tt * 2 + ic) * 2 + oc) * 128 + 128]
                                    for i in gsl:
                                        rhs = xt[:, i - g0, ic, dy:dy + HALF,
                                                 dx:dx + W]
                                        nc.tensor.matmul(
                                            ps[(i, oc)][:], lhs, rhs,
                                            start=(t == 0 and ic == 0),
                                            stop=(t == k2 - 1 and ic == 1))
                    for i in gsl:
                        st = opool.tile([128, 1024], mybir.dt.float32,
                                        tag="stage")
                        col = (slot_base + i) * 2
                        nc.scalar.add(st[:, 0:512], ps[(i, 0)][:],
                                      emb_t[:, col:col + 1])
                        nc.vector.tensor_scalar_add(
                            st[:, 512:1024], ps[(i, 1)][:],
                            emb_t[:, col + 1:col + 2])
                        q = nc.gpsimd if out_i % 2 == 0 else nc.sync
                        q.dma_start(out_d[slot_base + i], st[:])
                        out_i += 1
                slot_base += n

    nc.finalize()
    _PROGRAM_CACHE[key] = nc
    return nc


def _prepare(inputs, dt_key):
    """Host-side routing: build per-core in_maps + assembly metadata."""
    _, ndt = _DT_MAP[dt_key]
    x = np.asarray(inputs["x"], dtype=np.float32)
    y = np.asarray(inputs["y"]).astype(np.int64)
    arc = np.asarray(inputs["sample_arc"]).astype(np.int64)
    ws = [np.asarray(inputs[f"w{i}"], dtype=np.float32) for i in range(4)]
    es = [np.asarray(inputs[f"e{i}"], dtype=np.float32) for i in range(4)]
    B = x.shape[0]

    counts = np.bincount(arc, minlength=NUM_BRANCH)
    cfg = tuple((KERNEL_SIZES[b], int(math.ceil(2 * counts[b] / N_CORES)))
                for b in range(NUM_BRANCH))
    n_total = sum(n for _, n in cfg)

    # padded x: [B, 2, 128, H+6, W+6], channel-chunked, partition-major
    xp = np.zeros((B, 128, 2, H + 6, W + 6), dtype=np.float32)
    xr = x.reshape(B, 2, 128, H, W).transpose(0, 2, 1, 3, 4)
    xp[:, :, :, 3:3 + H, 3:3 + W] = xr

    # per-branch half-slot assignment: halves listed, padded to 8*n_b
    assign = {}  # b -> list of length 8*n_b of (sample, hh) or None
    for b in range(NUM_BRANCH):
        k, n = cfg[b]
        halves = [(s, hh) for s in range(B) if arc[s] == b for hh in (0, 1)]
        halves += [None] * (N_CORES * n - len(halves))
        assign[b] = halves

    # weights: shared across cores. [128, k2*2ic*2oc*128]
    w_arrs = {}
    for b in range(NUM_BRANCH):
        k, n = cfg[b]
        if n == 0:
            continue
        w6 = ws[b].reshape(2, 128, 2, 128, k, k)  # oc,m,ic,p,dy,dx
        wt = np.ascontiguousarray(w6.transpose(3, 4, 5, 2, 0, 1))
        w_arrs[b] = wt.reshape(128, k * k * 4 * 128).astype(ndt)

    in_maps = []
    meta = []  # per core: list over slot idx of (sample, hh) or None
    for core in range(N_CORES):
        im = {}
        slots = []
        emb_arr = np.zeros((128, n_total * 2), dtype=np.float32)
        idx = 0
        for b in range(NUM_BRANCH):
            k, n = cfg[b]
            if n == 0:
                continue
            c = k // 2
            rows, wp = HALF + 2 * c, W + 2 * c
            xa = np.zeros((128, n, 2, rows, wp), dtype=ndt)
            for i in range(n):
                hs = assign[b][core * n + i]
                slots.append(hs)
                if hs is not None:
                    s, hh = hs
                    r0 = hh * HALF + 3 - c
                    xa[:, i] = xp[s, :, :, r0:r0 + rows, 3 - c:3 - c + wp]
                    ev = es[b][y[s]]  # [256]
                    emb_arr[:, (idx + i) * 2 + 0] = ev[:128]
                    emb_arr[:, (idx + i) * 2 + 1] = ev[128:]
            im[f"x{b}"] = xa
            im[f"w{b}"] = w_arrs[b]
            idx += n
        im["emb"] = emb_arr
        in_maps.append(im)
        meta.append(slots)

    return cfg, in_maps, meta


def _assemble(results, meta, n_total):
    out = np.zeros((64, OUT_C, H, W), dtype=np.float32)
    for core in range(N_CORES):
        r = results[core]["out"]  # [n_total, 128, 1024] = [., p, oc*512]
        for idx, hs in enumerate(meta[core]):
            if hs is None:
                continue
            s, hh = hs
            blk = r[idx].reshape(128, 2, HALF, W).transpose(1, 0, 2, 3)
            out[s, :, hh * HALF:(hh + 1) * HALF, :] = \
                blk.reshape(OUT_C, HALF, W)
    return out


def run(inputs, trace=False, dt_key=None):
    if dt_key is None:
        dt_key = COMPUTE_DT
    _install_profile_hook()
    cfg, in_maps, meta = _prepare(inputs, dt_key)
    nc = _build_program(cfg, dt_key)
    res = bass_utils.run_bass_kernel_spmd(
        nc, in_maps, core_ids=list(range(N_CORES)), trace=trace)
    n_total = sum(n for _, n in cfg)
    out = _assemble(res.results, meta, n_total)
    return out, res


def kernel(**inputs):
    out, _ = run(inputs, trace=False)
    return out
